# revision 32
# baseline (speedup 1.0000x reference)
"""MoE layer (E=8 experts, top-2 routing) on 8 Trainium2 NeuronCores.

Expert-parallel sharding: core e holds expert e's weights (w1/w2/b1/b2).
Tokens are dispatched to the cores of their top-2 experts, each core runs
its expert's FFN on its gathered tokens and scales rows by the combine
weight, and the scaled contributions are summed back per token (the
all-to-all "return") to form the full output.

Shapes (hardcoded per the problem spec):
  x [2, 2048, 512] f32, router_w [8, 512], w1_all [8, 2048, 512],
  b1_all [8, 2048], w2_all [8, 512, 2048], b2_all [8, 512].
"""

import sys

sys.path.insert(0, "/opt/trn_rl_repo")

import numpy as np

import concourse.mybir as mybir
import concourse.tile as tile
from concourse import bacc

D_MODEL = 512
DFF = 2048
E = 8
K = 2
L = 2 * 2048  # total tokens
N_CORES = 8

FP = mybir.dt.float32

# Per-expert token capacity (padded). Expected load is L*K/E = 1024 with
# std ~30 under the near-uniform router; seed-0 max count is 1092. The
# program is built for the actual max count rounded up, so this is only
# the floor.
CAP = 1152

# float32r: single-pass fp32 matmul (full rate for moving dim >= 256),
# vs plain fp32 which runs as two half-speed passes.
MMDT = mybir.dt.float32r

_PROG_CACHE: dict = {}


def build_program(cap: int, bs: int = 512, warmup: int = 0, w1split: bool = False, xg_sync: bool = True, first_bs: int = 0, dve_bias: bool = True):
    """One SPMD program, run on all 8 cores; per-core data selects the expert.

    Per-core inputs:
      xgT  [512, cap]   gathered tokens for this expert, transposed (d-major)
      w1t  [512, 2048]  w1_e.T
      w2t  [2048, 512]  w2_e.T
      b1r  [16, 128, 1] b1_e
      b2r  [1, 512]     b2_e
      wgt  [cap//128, 128, 1] combine weights per gathered slot (0 for pads)
    Output:
      out  [cap, 512]   scaled expert contributions, row s = token slot s
    """
    nc = bacc.Bacc("TRN2", target_bir_lowering=False, debug=False)

    xgT = nc.dram_tensor("xgT", [D_MODEL, cap], MMDT, kind="ExternalInput")
    w1t = nc.dram_tensor("w1t", [D_MODEL, DFF], MMDT, kind="ExternalInput")
    w2t = nc.dram_tensor("w2t", [DFF, D_MODEL], MMDT, kind="ExternalInput")
    b1r = nc.dram_tensor("b1r", [DFF // 128, 128, 1], FP, kind="ExternalInput")
    b2r = nc.dram_tensor("b2r", [1, D_MODEL], MMDT, kind="ExternalInput")
    b2f = nc.dram_tensor("b2f", [128, D_MODEL], FP, kind="ExternalInput")
    wgt = nc.dram_tensor("wgt", [cap // 128, 128, 1], FP, kind="ExternalInput")
    onesr = nc.dram_tensor("onesr", [1, 128], MMDT, kind="ExternalInput")
    out = nc.dram_tensor("out", [cap, D_MODEL], FP, kind="ExternalOutput")

    KD = D_MODEL // 128  # 4 k-slices for mm1
    MD = DFF // 128  # 16 dff tiles
    # token blocks: widths >=256 keep f32r at full rate. An optional
    # smaller first block shrinks the bytes the first matmul group waits on.
    blocks = []
    off = 0
    if first_bs and cap > first_bs:
        blocks.append((0, first_bs))
        off = first_bs
    while off < cap:
        w = min(bs, cap - off)
        blocks.append((off, w))
        off += w

    with tile.TileContext(nc) as tc:
        with (
            tc.tile_pool(name="weights", bufs=1) as wpool,
            tc.tile_pool(name="h", bufs=2) as hpool,
            tc.tile_pool(name="psum", bufs=4, space="PSUM") as ppool,
            tc.tile_pool(name="outp", bufs=3) as opool,
            tc.tile_pool(name="consts", bufs=1) as cpool,
        ):
            # --- load inputs into SBUF ---
            # Order matters for the PE cold-start: mm1's operands (xgT, w1t)
            # first so matmuls start while w2t (only needed ~30us later)
            # still streams in. Issue on scalar/sync/gpsimd queues in
            # parallel so descriptor generation isn't serialized.
            xgT_sb = [
                wpool.tile([128, cap], MMDT, tag=f"xgT{k}", name=f"xgT_sb{k}")
                for k in range(KD)
            ]
            w1t_sb = [
                wpool.tile([128, DFF], MMDT, tag=f"w1t{k}", name=f"w1t_sb{k}")
                for k in range(KD)
            ]
            # Stream in mm1's operands in block-0-first order: the first
            # PSUM group (m=0, blk=0) needs xgT[k][:, :512] and
            # w1t[k][:, :128] for all k, i.e. ~1.3MB, not the full 6.5MB.
            b2_sb = ones_sb = None
            if warmup or not dve_bias:
                b2_sb = cpool.tile([1, D_MODEL], MMDT, tag="b2")
                nc.gpsimd.dma_start(out=b2_sb[:], in_=b2r[:, :])
                ones_sb = cpool.tile([1, 128], MMDT, tag="ones")
                nc.gpsimd.dma_start(out=ones_sb[:], in_=onesr[:, :])
            b0 = blocks[0][1]
            for k in range(KD):
                nc.scalar.dma_start(
                    out=xgT_sb[k][:, :b0], in_=xgT[k * 128 : (k + 1) * 128, :b0]
                )
            # first dff m-tiles in small chunks so group (m=0, blk=0) can
            # start after ~1.25MB; coarser chunks afterwards
            w1_chunks = [0, 128, 256, 512, 1024, 1536, 2048]
            for lo, hi in zip(w1_chunks, w1_chunks[1:]):
                for k in range(KD):
                    eng = nc.gpsimd if (w1split and k >= 2) else nc.sync
                    eng.dma_start(
                        out=w1t_sb[k][:, lo:hi],
                        in_=w1t[k * 128 : (k + 1) * 128, lo:hi],
                    )
            for off, w in blocks[1:]:
                for k in range(KD):
                    (nc.sync if xg_sync else nc.gpsimd).dma_start(
                        out=xgT_sb[k][:, off : off + w],
                        in_=xgT[k * 128 : (k + 1) * 128, off : off + w],
                    )
            b1_sb = wpool.tile([128, MD], FP, tag="b1")
            # b1r is [16,128,1]; lay tiles side by side: column m = tile m
            nc.gpsimd.dma_start(
                out=b1_sb[:], in_=b1r.rearrange("m p o -> p (m o)")
            )
            wgt_sb = wpool.tile([128, cap // 128], FP, tag="wgt")
            nc.gpsimd.dma_start(out=wgt_sb[:], in_=wgt.rearrange("u p o -> p (u o)"))
            b2f_sb = cpool.tile([128, D_MODEL], FP, tag="b2f")
            if dve_bias:
                nc.gpsimd.dma_start(out=b2f_sb[:], in_=b2f[:, :])
            w2t_sb = []
            for m in range(MD):
                t = wpool.tile([128, D_MODEL], MMDT, tag=f"w2t{m}")
                nc.sync.dma_start(out=t[:], in_=w2t[m * 128 : (m + 1) * 128, :])
                w2t_sb.append(t)

            # PE warm-up: ~5us of dummy matmuls on already-resident constants
            # while xgT/w1t stream in. HAM needs ~3.4us of sustained PE
            # activity to lift the 1.2GHz cold throttle; these run during the
            # DMA head so the first real matmuls start at 2.4GHz.
            if warmup:
                ps_w = ppool.tile([128, D_MODEL], FP, tag="ps2", name="ps_warm")
                for wi in range(warmup):
                    nc.tensor.matmul(
                        ps_w[:],
                        ones_sb[:],
                        b2_sb[:],
                        start=(wi == 0),
                        stop=(wi == warmup - 1),
                    )
                warm_sink = cpool.tile([1, 8], FP, tag="warm_sink")
                nc.vector.tensor_copy(warm_sink[:], ps_w[0:1, 0:8])

            # --- main loop over token blocks ---
            for off, ncols in blocks:
                # mm1: h_T[dff, tokens-in-block] = relu(w1t.T @ xgT + b1)
                h_sb = []
                for m in range(MD):
                    ps = ppool.tile([128, ncols], FP, tag="ps1")
                    for k in range(KD):
                        nc.tensor.matmul(
                            ps[:],
                            w1t_sb[k][:, m * 128 : (m + 1) * 128],
                            xgT_sb[k][:, off : off + ncols],
                            start=(k == 0),
                            stop=(k == KD - 1),
                        )
                    h = hpool.tile([128, ncols], MMDT, tag=f"h{m}", name=f"h_{m}")
                    if m % 2 == 0:
                        nc.scalar.activation(
                            h[:],
                            ps[:],
                            mybir.ActivationFunctionType.Relu,
                            bias=b1_sb[:, m : m + 1],
                        )
                    else:
                        # relu(x + b1) on DVE: (x add b1) max 0
                        nc.vector.tensor_scalar(
                            h[:],
                            ps[:],
                            b1_sb[:, m : m + 1],
                            0.0,
                            mybir.AluOpType.add,
                            mybir.AluOpType.max,
                        )
                    h_sb.append(h)
                # mm2: out[tok, d] = (h_T.T @ w2t) + b2, then scale rows
                for t in range(ncols // 128):
                    ps2 = ppool.tile([128, D_MODEL], FP, tag="ps2")
                    for m in range(MD):
                        nc.tensor.matmul(
                            ps2[:],
                            h_sb[m][:, t * 128 : (t + 1) * 128],
                            w2t_sb[m][:],
                            start=(m == 0),
                            stop=(dve_bias and m == MD - 1),
                        )
                    o = opool.tile([128, D_MODEL], FP, tag="o")
                    u = off // 128 + t  # token subtile index
                    if dve_bias:
                        # bias + gating scale on DVE: (ps2 + b2) * wgt
                        nc.vector.tensor_tensor(
                            out=o[:], in0=ps2[:], in1=b2f_sb[:],
                            op=mybir.AluOpType.add,
                        )
                        nc.vector.tensor_scalar_mul(o[:], o[:], wgt_sb[:, u : u + 1])
                    else:
                        # rank-1 bias: ones[1,128].T @ b2[1,512]
                        nc.tensor.matmul(
                            ps2[:],
                            ones_sb[:],
                            b2_sb[:],
                            start=False,
                            stop=True,
                        )
                        nc.vector.tensor_scalar_mul(o[:], ps2[:], wgt_sb[:, u : u + 1])
                    nc.scalar.dma_start(
                        out=out[u * 128 : (u + 1) * 128, :], in_=o[:]
                    )
    nc.compile()
    return nc


def _route(x_flat: np.ndarray, router_w: np.ndarray):
    """Host-side replica of the reference router: top-2 + renormalized weights."""
    logits = x_flat @ router_w.T  # [L, E]
    m = logits.max(axis=-1, keepdims=True)
    p = np.exp(logits - m)
    p /= p.sum(axis=-1, keepdims=True)
    order = np.argsort(-p, axis=-1)[:, :K]  # [L, K]
    pv = np.take_along_axis(p, order, axis=-1)
    pv = pv / (pv.sum(axis=-1, keepdims=True) + 1e-9)
    return order, pv


def _build_in_maps(x, router_w, w1_all, b1_all, w2_all, b2_all):
    """Shared staging: router + expert-parallel dispatch lists + per-core
    input maps. Returns (cap, in_maps, idx_lists)."""
    x_flat = np.asarray(x, np.float32).reshape(-1, D_MODEL)
    order, pv = _route(x_flat, np.asarray(router_w, np.float32))
    idx_lists, wgt_lists = [], []
    for e in range(E):
        sel = np.nonzero(order == e)
        idx_lists.append(sel[0])
        wgt_lists.append(pv[sel])
    max_n = max(len(t) for t in idx_lists)
    cap = max(CAP, -(-max_n // 128) * 128)
    in_maps = []
    for e in range(E):
        toks, ws = idx_lists[e], wgt_lists[e]
        n_e = len(toks)
        xg = np.zeros((cap, D_MODEL), np.float32)
        xg[:n_e] = x_flat[toks]
        wg = np.zeros((cap,), np.float32)
        wg[:n_e] = ws
        in_maps.append(
            {
                "xgT": np.ascontiguousarray(xg.T),
                "w1t": np.ascontiguousarray(np.asarray(w1_all, np.float32)[e].T),
                "w2t": np.ascontiguousarray(np.asarray(w2_all, np.float32)[e].T),
                "b1r": np.ascontiguousarray(
                    np.asarray(b1_all, np.float32)[e].reshape(DFF // 128, 128, 1)
                ),
                "b2r": np.asarray(b2_all, np.float32)[e].reshape(1, D_MODEL),
                "wgt": wg.reshape(cap // 128, 128, 1),
                "onesr": np.ones((1, 128), np.float32),
                "b2f": np.broadcast_to(
                    np.asarray(b2_all, np.float32)[e].reshape(1, D_MODEL), (128, D_MODEL)
                ).copy(),
            }
        )
    return cap, in_maps, idx_lists


def _get_program(cap: int):
    if cap not in _PROG_CACHE:
        _PROG_CACHE[cap] = build_program(cap)
    return _PROG_CACHE[cap]


def kernel(x, router_w, w1_all, b1_all, w2_all, b2_all):
    from concourse.bass_utils import run_bass_kernel_spmd

    x = np.asarray(x, dtype=np.float32)
    Bb, Nn, C = x.shape

    cap, in_maps, idx_lists = _build_in_maps(
        x, router_w, w1_all, b1_all, w2_all, b2_all
    )
    nc = _get_program(cap)

    res = run_bass_kernel_spmd(nc, in_maps, core_ids=list(range(N_CORES)))

    # Unshard: weighted all-to-all return == scatter-add contributions per token.
    final = np.zeros((Bb * Nn, C), np.float32)
    for e in range(E):
        toks = idx_lists[e]
        final[toks] += res.results[e]["out"][: len(toks)]
    return final.reshape(Bb, Nn, C)


def time_kernel(x, router_w, w1_all, b1_all, w2_all, b2_all, iters: int = 50):
    """Wall-clock the NEFF execution: jit once, device-put inputs, run a
    pipelined loop. Returns estimated ns per execution (all 8 cores)."""
    import time as _time

    import jax
    from jax.experimental.shard_map import shard_map
    from jax.sharding import Mesh, NamedSharding, PartitionSpec

    from concourse import bass2jax

    cap, in_maps, _ = _build_in_maps(x, router_w, w1_all, b1_all, w2_all, b2_all)
    nc = _get_program(cap)

    bass2jax.install_neuronx_cc_hook()

    import concourse.mybir as _mb

    partition_name = nc.partition_id_tensor.name if nc.partition_id_tensor else None
    in_names, out_names, out_avals, zero_outs = [], [], [], []
    for alloc in nc.m.functions[0].allocations:
        if not isinstance(alloc, _mb.MemoryLocationSet):
            continue
        name = alloc.memorylocations[0].name
        if alloc.kind == "ExternalInput":
            if name != partition_name:
                in_names.append(name)
        elif alloc.kind == "ExternalOutput":
            shape = tuple(alloc.tensor_shape)
            dtype = _mb.dt.np(alloc.dtype)
            out_names.append(name)
            out_avals.append(jax.core.ShapedArray(shape, dtype))
            zero_outs.append(np.zeros(shape, dtype))
    n_params = len(in_names)
    all_in_names = list(in_names) + list(out_names)
    if partition_name is not None:
        all_in_names.append(partition_name)
    if nc.dbg_addr is not None:
        extra_dbg = {nc.dbg_addr.name: np.zeros((1, 2), np.uint32)}
        in_maps = [{**m, **extra_dbg} for m in in_maps]

    def _body(*args):
        operands = list(args)
        if partition_name is not None:
            operands.append(bass2jax.partition_id_tensor())
        outs = bass2jax._bass_exec_p.bind(
            *operands,
            out_avals=tuple(out_avals),
            in_names=tuple(all_in_names),
            out_names=tuple(out_names),
            lowering_input_output_aliases=(),
            sim_require_finite=True,
            sim_require_nnan=True,
            nc=nc,
        )
        return tuple(outs)

    devices = jax.devices()[:N_CORES]
    mesh = Mesh(np.asarray(devices), ("core",))
    spec = PartitionSpec("core")
    in_specs = (spec,) * (n_params + len(out_names))
    out_specs = (spec,) * len(out_names)
    fn = jax.jit(
        shard_map(_body, mesh=mesh, in_specs=in_specs, out_specs=out_specs,
                  check_rep=False),
        keep_unused=True,
    )
    sharding = NamedSharding(mesh, spec)
    concat_in = [
        jax.device_put(
            np.concatenate([np.asarray(in_maps[c][n]) for c in range(N_CORES)], axis=0),
            sharding,
        )
        for n in in_names[:n_params]
    ]
    concat_zeros = [
        jax.device_put(
            np.zeros((N_CORES * z.shape[0], *z.shape[1:]), z.dtype), sharding
        )
        for z in zero_outs
    ]
    # warmup + compile
    outs = fn(*concat_in, *concat_zeros)
    jax.block_until_ready(outs)

    t0 = _time.perf_counter()
    for _ in range(iters):
        outs = fn(*concat_in, *concat_zeros)
    jax.block_until_ready(outs)
    dt = _time.perf_counter() - t0
    return dt / iters * 1e9
